# revision 1
# baseline (speedup 1.0000x reference)
"""Trainium2 (8 NeuronCores) kernel for nn_AttentionEdgeWeighting.

out[e] = message[e] * softmax_over_edges_with_same_target(
             leaky_relu(message[e] @ w_m[h] + x_e[target[e]] @ w_x[h], 0.1))

Strategy (the sharding hint's "pre-partition edges by target node" option):
  * Host: sort edges by target, split the node range into 8 contiguous
    chunks with balanced edge counts (one per core), pack each core's
    nodes greedily into fixed-shape "windows" (<=127 nodes, <=17*128
    edges, edge-padded).  Every window's scatter-softmax closes locally,
    so the 8 cores run fully independently - no collectives.
  * Device (per window): scores via TensorE matmuls against
    block-diagonal weights on host-transposed bf16 messages; one-hot
    (edge,node) matrices from iota-compares; segment sums and gathers as
    one-hot matmuls through PSUM; exp(leaky_relu(x)) computed as
    exp(0.1x)*exp(0.9*relu(x)); final message*alpha via selector-matmul
    broadcast + DVE multiply.  Softmax max-subtraction is dropped:
    scores are bounded (|s| < ~25) so fp32 exp cannot overflow and the
    result is mathematically identical.
"""
import numpy as np
import ml_dtypes

from concourse.bass_utils import run_bass_kernel_spmd

bf = ml_dtypes.bfloat16
H = 8
D = 32
FDIM = 256
NC = 8
NODE_CAP = 127

from contextlib import ExitStack

import concourse.bass as bass
from concourse import mybir

BF = mybir.dt.bfloat16
F32 = mybir.dt.float32

T = 15                 # tiles per window
EC = T * 128           # edge columns per window (1920)
NSLICE = [512, 512, 512, 384]
assert sum(NSLICE) == EC
SB = []
_o = 0
for _n in NSLICE:
    SB.append((_o, _o + _n))
    _o += _n

N_CONST = 9
NK = 2 * len(NSLICE)     # out-multiply slice count


class Sched:
    def __init__(self):
        self.counts = {}
        self.events = {}

    def reg(self, evt, sem, inc):
        self.counts[sem] = self.counts.get(sem, 0) + inc
        self.events[evt] = (sem, self.counts[sem])

    def get(self, evt):
        return self.events[evt]


def plan_schedule(W, phase=8):
    s = Sched()
    for i in range(N_CONST):
        s.reg(("const", i), "s_const", 16)
    for w in range(W):
        s.reg(("mt", w), f"s_mt{w % 2}", 16)
        s.reg(("xt", w), f"s_xt{w % 2}", 16)
        s.reg(("tr", w), f"s_tr{w % 2}", 16)
    for w in range(W):
        if phase >= 2:
            s.reg(("o", w), "s_dve", 1)
            s.reg(("ot", w), "s_dve", 1)
        if phase >= 4:
            s.reg(("extm", w), "s_dve", 1)
        if phase >= 5:
            s.reg(("excopy", w), "s_dve", 1)
        if phase >= 6:
            s.reg(("ismax", w), "s_dve", 1)
            s.reg(("isrec", w), "s_dve", 1)
            s.reg(("spost", w), "s_dve", 1)
        if phase >= 7:
            s.reg(("at", w), "s_dve", 1)
        if phase >= 8:
            for k in range(NK):
                s.reg(("outmul", w, k), "s_dve", 1)
    for w in range(W):
        if phase >= 3:
            s.reg(("xscopy1", w), "s_act", 1)
            s.reg(("xscopy2", w), "s_act", 1)
        if phase >= 4:
            s.reg(("relu9", w), "s_act", 1)
            s.reg(("exp1", w), "s_act", 1)
            s.reg(("exp9", w), "s_act", 1)
        s.reg(("dmaout", w), f"s_out{w % 2}", 16)
    for w in range(W):
        s.reg(("trbmm", w), "s_pe", 1)
        if phase >= 3:
            s.reg(("xs", w), "s_pe", 1)
            s.reg(("xst", w), "s_pe", 1)
            s.reg(("xg", w), "s_pe", 1)
        if phase >= 5:
            s.reg(("transp", w), "s_pe", 1)
        if phase >= 6:
            s.reg(("seg", w), "s_pe", 1)
        if phase >= 7:
            s.reg(("g2", w), "s_pe", 1)
        if phase >= 8:
            for k in range(NK):
                s.reg(("bcast", w, k), "s_pe", 1)
    return s


def build_kernel(W, phase=8):
    nc = bass.Bass()
    d_msg = nc.declare_dram_parameter("msgT", [W, 256, EC], BF, isOutput=False)
    d_x = nc.declare_dram_parameter("xT", [W, 128, 256], BF, isOutput=False)
    d_tc = nc.declare_dram_parameter("tgt_col", [W, 128, T], BF, isOutput=False)
    d_tr = nc.declare_dram_parameter("tgt_row", [W, 1, EC], BF, isOutput=False)
    d_wm = nc.declare_dram_parameter("wm", [2, 128, 8], BF, isOutput=False)
    d_wx = nc.declare_dram_parameter("wx", [2, 128, 8], BF, isOutput=False)
    d_sel = nc.declare_dram_parameter("sel", [2, 8, 128], BF, isOutput=False)
    d_ic = nc.declare_dram_parameter("iota_col", [128, 1], F32, isOutput=False)
    d_im = nc.declare_dram_parameter("iota_mat_bf", [128, 128], BF, isOutput=False)
    d_id = nc.declare_dram_parameter("identity", [128, 128], F32, isOutput=False)
    d_idb = nc.declare_dram_parameter("identity_bf", [128, 128], BF, isOutput=False)
    d_ones = nc.declare_dram_parameter("ones_bf", [1, 128], BF, isOutput=False)
    d_out = nc.declare_dram_parameter("outT", [W, 256, EC], BF, isOutput=True)

    sched = plan_schedule(W, phase)

    ctx = ExitStack()
    sb = lambda nm, shape, dt: ctx.enter_context(nc.sbuf_tensor(nm, shape, dt))
    MT = [sb(f"MT{i}", [128, 2 * EC], BF) for i in range(2)]
    XT = [sb(f"XTb{i}", [128, 256], BF) for i in range(2)]
    TR = [sb(f"TRb{i}", [1, EC], BF) for i in range(2)]
    O = [sb(f"Ob{i}", [128, EC], BF) for i in range(2)]
    OT = [sb(f"OTb{i}", [128, EC], BF) for i in range(2)]
    R9 = [sb(f"R9b{i}", [8, EC], F32) for i in range(2)]
    E1 = [sb(f"E1b{i}", [8, EC], F32) for i in range(2)]
    E2 = [sb(f"E2b{i}", [8, EC], F32) for i in range(2)]
    EXT = [sb(f"EXTb{i}", [8, EC], BF) for i in range(2)]
    EX = [sb(f"EXb{i}", [128, 8 * T], BF) for i in range(2)]
    XSB = [sb(f"XSBb{i}", [8, 128], F32) for i in range(2)]
    XJ = [sb(f"XJb{i}", [128, 8], BF) for i in range(2)]
    ISF = [sb(f"ISFb{i}", [128, 8], F32) for i in range(2)]
    ISR = [sb(f"ISRb{i}", [128, 8], F32) for i in range(2)]
    ISB = [sb(f"ISBb{i}", [128, 8], BF) for i in range(2)]
    AT = [sb(f"ATb{i}", [8, EC], BF) for i in range(2)]
    OUTB = [sb(f"OUTBb{i}", [128, 2 * EC], BF) for i in range(2)]
    TC = sb("TCc", [128, W * T], BF)
    WM = sb("WMc", [128, 16], BF)
    WX = sb("WXc", [128, 16], BF)
    SEL = sb("SELc", [8, 256], BF)
    ICOL = sb("ICOLc", [128, 1], F32)
    IMAT = sb("IMATc", [128, 128], BF)
    IDEN = sb("IDENc", [128, 128], F32)
    IDENB = sb("IDENBc", [128, 128], BF)
    ONES = sb("ONESc", [1, 128], BF)

    big5 = ctx.enter_context(nc.psum_tensor("big5", [128, 2048], F32))
    misc = ctx.enter_context(nc.psum_tensor("miscp", [128, 512], F32))
    AB = ctx.enter_context(nc.psum_tensor("ABp", [128, 1024], F32))
    EXTPB = ctx.enter_context(nc.psum_tensor("extpb", [128, 136], BF))
    SC = big5[0:8, 0:EC]                      # score, later G2 (inv_s gather)
    TRB_P = big5[0:128, 0:EC]                 # target-row broadcast (pre-score)
    XS_P = misc[0:8, 0:128]
    XST_P = misc[0:128, 128:136]
    ST_P = misc[0:128, 136:144]
    EXTP = EXTPB[0:128, 0:8 * T]

    sems = {}
    for name in ["s_const", "s_mt0", "s_mt1", "s_xt0", "s_xt1", "s_tr0",
                 "s_tr1", "s_out0", "s_out1", "s_dve", "s_pe", "s_act"]:
        sems[name] = ctx.enter_context(nc.semaphore(name))

    DMA_SEMS = ("s_const", "s_mt0", "s_mt1", "s_xt0", "s_xt1", "s_tr0",
                "s_tr1", "s_out0", "s_out1")

    def wait(eng, evt):
        if evt not in sched.events:
            return
        sem, cnt = sched.get(evt)
        eng.wait_ge(sems[sem], cnt)

    def inc(inst, evt):
        sem, _ = sched.get(evt)
        inst.then_inc(sems[sem], 16 if sem in DMA_SEMS else 1)
        return inst

    EQ = mybir.AluOpType.is_equal
    MUL = mybir.AluOpType.mult
    MAX = mybir.AluOpType.max

    with nc.Block() as block:

        @block.sync
        def _(sync):
            inc(sync.dma_start(TC[:].rearrange("p (w t) -> p w t", t=T),
                               d_tc[:].rearrange("w p t -> p w t")), ("const", 0))
            inc(sync.dma_start(WM[:].rearrange("p (g h) -> p g h", g=2),
                               d_wm[:].rearrange("g p h -> p g h")), ("const", 1))
            inc(sync.dma_start(WX[:].rearrange("p (g h) -> p g h", g=2),
                               d_wx[:].rearrange("g p h -> p g h")), ("const", 2))
            inc(sync.dma_start(SEL[:].rearrange("h (g f) -> h g f", g=2),
                               d_sel[:].rearrange("g h f -> h g f")), ("const", 3))
            inc(sync.dma_start(ICOL[:], d_ic[:]), ("const", 4))
            inc(sync.dma_start(IMAT[:], d_im[:]), ("const", 5))
            inc(sync.dma_start(IDEN[:], d_id[:]), ("const", 6))
            inc(sync.dma_start(IDENB[:], d_idb[:]), ("const", 7))
            inc(sync.dma_start(ONES[:], d_ones[:]), ("const", 8))
            for w in range(W):
                b = w % 2
                if w >= 2:
                    wait(sync, ("outmul", w - 2, NK - 1))   # MT buf free
                inc(sync.dma_start(MT[b][:].rearrange("p (g e) -> p g e", g=2),
                                   d_msg[w].rearrange("(g p) e -> p g e", p=128)),
                    ("mt", w))
                if w >= 2:
                    wait(sync, ("xs", w - 2))          # XT buf free
                inc(sync.dma_start(XT[b][:], d_x[w]), ("xt", w))
                if w >= 2:
                    wait(sync, ("trbmm", w - 2))       # TR buf free
                inc(sync.dma_start(TR[b][:], d_tr[w]), ("tr", w))

        @block.vector
        def _(dve):
            for w in range(W):
                b = w % 2
                if w == 0:
                    wait(dve, ("const", 8))
                if w >= 2:
                    wait(dve, ("seg", w - 2))          # O buf free
                tc_b = TC[:, w * T:(w + 1) * T].to_broadcast((128, T, 128))
                im_b = IMAT[:].rearrange("p (a j) -> p a j", a=1).to_broadcast((128, T, 128))
                if phase >= 2:
                    inc(dve.tensor_tensor(O[b][:].rearrange("p (t j) -> p t j", t=T),
                                          tc_b, im_b, EQ), ("o", w))
                    wait(dve, ("trbmm", w))
                    if w >= 2:
                        wait(dve, ("g2", w - 2))           # OT buf free
                    inc(dve.tensor_scalar(OT[b][:], TRB_P[:], ICOL[:], None, EQ), ("ot", w))
                if phase >= 4:
                    wait(dve, ("exp9", w))
                    wait(dve, ("exp1", w))
                    if w >= 2:
                        wait(dve, ("transp", w - 2))       # EXT buf free (PE reader)
                        wait(dve, ("bcast", w - 2, NK - 1))
                    inc(dve.tensor_tensor(EXT[b][:], E1[b][:], E2[b][:], MUL),
                        ("extm", w))
                if phase >= 5:
                    wait(dve, ("transp", w))
                    inc(dve.tensor_copy(EX[b][:], EXTP[:]), ("excopy", w))
                if phase >= 6:
                    wait(dve, ("seg", w))
                    if w >= 2:
                        wait(dve, ("g2", w - 2))           # ISB buf free
                    inc(dve.tensor_scalar(ISF[b][:], ST_P[:], 1e-16, None, MAX), ("ismax", w))
                    wait(dve, ("ismax", w))
                    inc(dve.reciprocal(ISR[b][:], ISF[b][:]), ("isrec", w))
                    wait(dve, ("isrec", w))
                    inc(dve.tensor_copy(ISB[b][:], ISR[b][:]), ("spost", w))
                if phase >= 7:
                    wait(dve, ("g2", w))
                    if w >= 2:
                        wait(dve, ("bcast", w - 2, NK - 1))  # AT buf free
                    inc(dve.tensor_tensor(AT[b][:], EXT[b][:], SC, MUL), ("at", w))
                if phase >= 8:
                    if w >= 2:
                        wait(dve, ("dmaout", w - 2))       # OUTB buf free
                    for k in range(NK):
                        hf, si = divmod(k, len(NSLICE))
                        o0, o1 = SB[si]
                        wait(dve, ("bcast", w, k))
                        ab = AB[:, (k % 2) * 512:(k % 2) * 512 + (o1 - o0)]
                        inc(dve.tensor_tensor(OUTB[b][:, hf * EC + o0: hf * EC + o1],
                                              MT[b][:, hf * EC + o0: hf * EC + o1],
                                              ab, MUL), ("outmul", w, k))

        @block.scalar
        def _(act):
            for w in range(W):
                b = w % 2
                if phase >= 3:
                    wait(act, ("xs", w))
                    if w >= 2:
                        wait(act, ("xst", w - 2))          # XSB buf free
                    inc(act.copy(XSB[b][:], XS_P[:]), ("xscopy1", w))
                    wait(act, ("xst", w))
                    if w >= 2:
                        wait(act, ("xg", w - 2))           # XJ buf free
                    inc(act.copy(XJ[b][:], XST_P[:]), ("xscopy2", w))
                if phase >= 4:
                    wait(act, ("xg", w))
                    if w >= 2:
                        wait(act, ("extm", w - 2))         # E1/E2/R9 bufs free
                    inc(act.activation(R9[b][:], SC,
                                       mybir.ActivationFunctionType.Relu,
                                       scale=0.9), ("relu9", w))
                    inc(act.activation(E1[b][:], SC,
                                       mybir.ActivationFunctionType.Exp,
                                       scale=0.1), ("exp1", w))
                    wait(act, ("relu9", w))
                    inc(act.activation(E2[b][:], R9[b][:],
                                       mybir.ActivationFunctionType.Exp), ("exp9", w))
                if phase >= 8:
                    wait(act, ("outmul", w, NK - 1))
                else:
                    wait(act, ("mt", w))
                inc(act.dma_start(d_out[w].rearrange("(g p) e -> p g e", p=128),
                                  OUTB[b][:].rearrange("p (g e) -> p g e", g=2)),
                    ("dmaout", w))

        @block.tensor
        def _(pe):
            for w in range(W):
                b = w % 2
                if w == 0:
                    wait(pe, ("const", 8))
                # target-row broadcast into psum (K=1 ones matmul)
                wait(pe, ("tr", w))
                if w >= 1:
                    wait(pe, ("at", w - 1))            # big5 free (G2 consumed)
                for i, (o0, o1) in enumerate(SB):
                    mm = pe.matmul(big5[0:128, o0:o1], ONES[:], TR[b][0:1, o0:o1],
                                   start=True, stop=True, skip_group_check=True)
                    if i == len(SB) - 1:
                        inc(mm, ("trbmm", w))
                if phase >= 3:
                    wait(pe, ("xt", w))
                    if w >= 1:
                        wait(pe, ("xscopy1", w - 1))       # XS_P psum free
                    pe.matmul(XS_P, WX[:, 0:8], XT[b][:, 0:128], start=True, stop=False,
                              skip_group_check=True)
                    inc(pe.matmul(XS_P, WX[:, 8:16], XT[b][:, 128:256], start=False,
                                  stop=True, skip_group_check=True), ("xs", w))
                    wait(pe, ("mt", w))
                    wait(pe, ("ot", w))
                    for hf in range(2):
                        for o0, o1 in SB:
                            pe.matmul(big5[0:8, o0:o1], WM[:, hf * 8:hf * 8 + 8],
                                      MT[b][:, hf * EC + o0: hf * EC + o1],
                                      start=(hf == 0), stop=False, skip_group_check=True)
                    wait(pe, ("xscopy1", w))
                    if w >= 1:
                        wait(pe, ("xscopy2", w - 1))       # XST_P psum free
                    inc(pe.transpose(XST_P[:], XSB[b][:], IDEN[0:8, 0:8]), ("xst", w))
                    wait(pe, ("xscopy2", w))
                    for i, (o0, o1) in enumerate(SB):
                        mm = pe.matmul(big5[0:8, o0:o1], XJ[b][:], OT[b][:, o0:o1],
                                       start=False, stop=True, skip_group_check=True)
                        if i == len(SB) - 1:
                            inc(mm, ("xg", w))
                if phase >= 5:
                    wait(pe, ("extm", w))
                    if w >= 1:
                        wait(pe, ("excopy", w - 1))        # EXTP psum free
                    for t in range(T):
                        mm = pe.transpose(EXTP[:, t * 8:(t + 1) * 8],
                                          EXT[b][0:8, t * 128:(t + 1) * 128], IDENB[0:8, 0:8])
                        if t == T - 1:
                            inc(mm, ("transp", w))
                if phase >= 6:
                    wait(pe, ("o", w))
                    wait(pe, ("excopy", w))
                    if w >= 1:
                        wait(pe, ("spost", w - 1))         # ST_P psum free
                    for t in range(T):
                        mm = pe.matmul(ST_P[:], O[b][:, t * 128:(t + 1) * 128],
                                       EX[b][:, t * 8:(t + 1) * 8],
                                       start=(t == 0), stop=(t == T - 1),
                                       skip_group_check=True)
                        if t == T - 1:
                            inc(mm, ("seg", w))
                if phase >= 7:
                    wait(pe, ("spost", w))
                    wait(pe, ("exp1", w))                  # SC free after ACT reads
                    for i, (o0, o1) in enumerate(SB):
                        mm = pe.matmul(big5[0:8, o0:o1], ISB[b][:], OT[b][:, o0:o1],
                                       start=True, stop=True, skip_group_check=True)
                        if i == len(SB) - 1:
                            inc(mm, ("g2", w))
                if phase >= 8:
                    wait(pe, ("at", w))
                    for k in range(NK):
                        hf, si = divmod(k, len(NSLICE))
                        o0, o1 = SB[si]
                        if k >= 2:
                            wait(pe, ("outmul", w, k - 2))
                        elif w >= 1:
                            wait(pe, ("outmul", w - 1, NK - 2 + k))
                        inc(pe.matmul(AB[:, (k % 2) * 512:(k % 2) * 512 + (o1 - o0)],
                                      SEL[0:8, hf * 128:(hf + 1) * 128],
                                      AT[b][0:8, o0:o1],
                                      start=True, stop=True, skip_group_check=True),
                            ("bcast", w, k))

    return nc, ctx


def _pack_windows(counts, lo, hi):
    wins = []
    n = lo
    while n < hi:
        n0, e = n, 0
        while n < hi and (n - n0) < NODE_CAP and e + counts[n] <= EC:
            e += counts[n]
            n += 1
        assert n > n0
        wins.append((n0, n, e))
    return wins


def _build_plan(target, num_nodes):
    E = target.shape[0]
    counts = np.bincount(target, minlength=num_nodes)
    cum = np.cumsum(counts)
    bounds = [0]
    for c in range(1, NC):
        bounds.append(int(np.searchsorted(cum, E * c / NC)))
    bounds.append(num_nodes)
    order = np.argsort(target, kind="stable")
    row_start = np.zeros(num_nodes + 1, dtype=np.int64)
    row_start[1:] = cum
    plans = [_pack_windows(counts, bounds[c], bounds[c + 1]) for c in range(NC)]
    return {"order": order, "tsort": target[order], "row_start": row_start,
            "plans": plans, "W": max(len(p) for p in plans)}


def _build_core_inputs(plan, message, x_e, weight):
    W = plan["W"]
    order, row_start = plan["order"], plan["row_start"]
    num_nodes = x_e.shape[0]
    w = np.asarray(weight, np.float32)
    Wm = np.zeros((2, 128, H), np.float32)
    Wx = np.zeros((2, 128, H), np.float32)
    for h in range(H):
        half, fo = divmod(h * D, 128)
        Wm[half, fo:fo + D, h] = w[h, :D]
        Wx[half, fo:fo + D, h] = w[h, D:]
    SEL = np.zeros((2, H, 128), np.float32)
    for half in range(2):
        for f in range(128):
            SEL[half, (half * 128 + f) // D, f] = 1.0
    consts = {
        "wm": Wm.astype(bf), "wx": Wx.astype(bf), "sel": SEL.astype(bf),
        "iota_col": np.arange(128, dtype=np.float32).reshape(128, 1),
        "iota_mat_bf": np.broadcast_to(np.arange(128, dtype=np.float32),
                                       (128, 128)).astype(bf).copy(),
        "identity": np.eye(128, dtype=np.float32),
        "identity_bf": np.eye(128, dtype=np.float32).astype(bf),
        "ones_bf": np.ones((1, 128), np.float32).astype(bf),
    }
    in_maps, meta = [], []
    for c in range(NC):
        wins = plan["plans"][c]
        msgT = np.zeros((W, FDIM, EC), bf)
        xT = np.zeros((W, 128, 256), bf)
        tgt = np.full((W, EC), NODE_CAP, np.float32)
        eids = np.full((W, EC), -1, np.int64)
        for wi, (n0, n1, ne) in enumerate(wins):
            e0 = row_start[n0]
            ids = order[e0:e0 + ne]
            msgT[wi, :, :ne] = np.asarray(message[ids], np.float32).T.astype(bf)
            tgt[wi, :ne] = (plan["tsort"][e0:e0 + ne] - n0).astype(np.float32)
            eids[wi, :ne] = ids
            nn = min(128, num_nodes - n0)
            xw = np.zeros((128, FDIM), np.float32)
            xw[:nn] = np.asarray(x_e[n0:n0 + nn], np.float32)
            xT[wi, :, 0:128] = xw[:, 0:128].T.astype(bf)
            xT[wi, :, 128:256] = xw[:, 128:256].T.astype(bf)
        tgt_col = tgt.reshape(W, T, 128).transpose(0, 2, 1)
        m = {"msgT": msgT, "xT": xT,
             "tgt_col": np.ascontiguousarray(tgt_col).astype(bf),
             "tgt_row": tgt.astype(bf).reshape(W, 1, EC)}
        m.update(consts)
        in_maps.append(m)
        meta.append(eids)
    return in_maps, meta


def kernel(source, target, message, x_e, weight):
    target = np.asarray(target)
    tgt_i = target.astype(np.int64)
    message = np.asarray(message)
    x_e = np.asarray(x_e)
    weight = np.asarray(weight)
    E = message.shape[0]

    plan = _build_plan(tgt_i, x_e.shape[0])
    in_maps, meta = _build_core_inputs(plan, message, x_e, weight)
    nc, ctx = build_kernel(plan["W"])
    res = run_bass_kernel_spmd(nc, in_maps, core_ids=list(range(NC)))
    ctx.close()

    out = np.zeros((E, FDIM), np.float32)
    for c in range(NC):
        o = np.asarray(res.results[c]["outT"], np.float32)
        eids = meta[c]
        mask = eids >= 0
        out[eids[mask]] = o.transpose(0, 2, 1)[mask]
    return out

